# revision 5
# baseline (speedup 1.0000x reference)
"""Windowed attention with dynamic position bias — Trainium2 Bass kernel.

Problem shapes (hardcoded): qkv (3,4,32768,192) f32, H=128, W=256, C=192,
HEADS=6, hd=32, windows 8x32 -> N=256 tokens, nW=128 windows, B=4.

Sharding: 8 cores, each takes 16 consecutive windows (= 16 H-rows of the
image) across all 4 batch elements. mask is sharded by window; the tiny
pos-bias MLP runs on host (0.004% of FLOPs) and exp(rpb)/exp(mask) factors
ship as fp16.

Device math per (b, w, head):
  S^T[m,n] = sum_d k[m,d] q[n,d] * scale        (PE, fp32, K=32)
  P^T      = exp(S^T) * exp(mask^T + rpb^T)*2^-4 (ACT exp PSUM->SBUF fp16,
                                                  DVE 16-bit mult, 2x mode)
  O[n,d]   = sum_m P^T[m,n] v_aug[m,d]          (PE fp16 FWL; col 32 of
                                                 v_aug is ones -> softmax
                                                 denominators for free)
  out      = O[:, :32] / O[:, 32]               (DVE reciprocal + mult)
The 2^-4 prescale keeps exp products in fp16 range; it cancels in the
normalization.
"""

import numpy as np

HSP, WSP = 8, 32
HEADS = 6
HD = 32
N = HSP * WSP  # 256
B = 4
H_FULL, W_FULL, C = 128, 256, 192
N_CORES = 8
W_PER_CORE = 16  # windows per core
EPS = 1e-5
PRESCALE = 0.0625  # 2^-4
SCALE = HD ** -0.5

_NC_CACHE = {}


def _pos_mlp_host(rpe, pw0, pb0, g1, be1, w1, b1, g2, be2, w2, b2, g3, be3, w3, b3):
    def ln(x, g, b):
        m = x.mean(-1, keepdims=True)
        v = ((x - m) ** 2).mean(-1, keepdims=True)
        return (x - m) / np.sqrt(v + EPS) * g + b

    x = rpe @ pw0.T + pb0
    x = np.maximum(ln(x, g1, be1), 0.0) @ w1.T + b1
    x = np.maximum(ln(x, g2, be2), 0.0) @ w2.T + b2
    x = np.maximum(ln(x, g3, be3), 0.0) @ w3.T + b3
    return x  # (945, HEADS)


def _build_nc():
    import concourse.bass as bass
    import concourse.bacc as bacc
    import concourse.tile as tile
    from concourse import mybir

    f32 = mybir.dt.float32
    f16 = mybir.dt.float16
    AF = mybir.ActivationFunctionType

    nc = bacc.Bacc("TRN2", target_bir_lowering=False, debug=False)
    qk_main_d = nc.dram_tensor("qk_main", (W_PER_CORE, B, 128, 2, 256), f16,
                               kind="ExternalInput")
    qk_tail_d = nc.dram_tensor("qk_tail", (W_PER_CORE, 2, 128, 2, 256), f16,
                               kind="ExternalInput")
    vaug_d = nc.dram_tensor("v_aug", (W_PER_CORE, B, 128, 2, 198), f16,
                            kind="ExternalInput")
    emt_d = nc.dram_tensor("emt", (W_PER_CORE, 128, 2, 256), f16,
                           kind="ExternalInput")
    ert_d = nc.dram_tensor("ert", (128, 6, 2, 256), f16, kind="ExternalInput")
    out_d = nc.dram_tensor("out", (B, W_PER_CORE, 128, 2, 192), f32,
                           kind="ExternalOutput")

    def bcast_ap(t_ap, extra):
        # insert a [0, count] dim after partition dim of a 2d-view AP
        return bass.AP(tensor=t_ap.tensor, offset=t_ap.offset,
                       ap=[t_ap.ap[0], extra] + list(t_ap.ap[1:]))

    with tile.TileContext(nc) as tc:
        with (
            tc.tile_pool(name="singles", bufs=1) as singles,
            tc.tile_pool(name="emtp", bufs=2) as emtp,
            tc.tile_pool(name="emrp", bufs=2) as emrp,
            tc.tile_pool(name="qkp", bufs=3) as qkp,
            tc.tile_pool(name="tailp", bufs=2) as tailp,
            tc.tile_pool(name="vp", bufs=3) as vp,
            tc.tile_pool(name="pp", bufs=3) as pp,
            tc.tile_pool(name="p2p", bufs=3) as p2p,
            tc.tile_pool(name="recp", bufs=3) as recp,
            tc.tile_pool(name="outp", bufs=3) as outp,
            tc.tile_pool(name="spsum", bufs=2, space="PSUM") as spsum,
            tc.tile_pool(name="opsum", bufs=2, space="PSUM") as opsum,
        ):
            ert_t = singles.tile([128, 6, 2, 256], f16)
            nc.default_dma_engine.dma_start(out=ert_t[:], in_=ert_d[:])

            for w in range(W_PER_CORE):
                emt_t = emtp.tile([128, 2, 256], f16)
                nc.default_dma_engine.dma_start(out=emt_t[:], in_=emt_d[w])
                # EMR[w] = exp(mask^T)*2^-4 * exp(rpb^T) for all 6 heads
                emr_t = emrp.tile([128, 6, 2, 256], f16)
                nc.vector.tensor_mul(
                    emr_t[:],
                    bcast_ap(emt_t[:], [0, 6]),
                    ert_t[:],
                )
                for bp in range(2):
                    tail_t = tailp.tile([128, 2, 256], f16)
                    nc.default_dma_engine.dma_start(out=tail_t[:],
                                                    in_=qk_tail_d[w, bp])
                    for bl in range(2):
                        b = bp * 2 + bl
                        qkm_t = qkp.tile([128, 2, 256], f16)
                        nc.default_dma_engine.dma_start(out=qkm_t[:],
                                                        in_=qk_main_d[w, b])
                        v_t = vp.tile([128, 2, 198], f16)
                        nc.default_dma_engine.dma_start(out=v_t[:],
                                                        in_=vaug_d[w, b])
                        o_t = opsum.tile([128, 512], f32)
                        for g in range(2):  # head groups of 3
                            s_t = spsum.tile([128, 1536], f32)
                            for hl in range(3):
                                h = g * 3 + hl
                                if h < 4:
                                    q_ap = qkm_t[h * 32:(h + 1) * 32, 0]
                                    k_src = qkm_t
                                    kp0 = h * 32
                                    k_col = 1
                                else:
                                    p0 = bl * 64 + (h - 4) * 32
                                    q_ap = tail_t[p0:p0 + 32, 0]
                                    k_src = tail_t
                                    kp0 = p0
                                    k_col = 1
                                for mt in range(2):
                                    nc.tensor.matmul(
                                        s_t[:, hl * 512 + mt * 256:
                                            hl * 512 + mt * 256 + 256],
                                        k_src[kp0:kp0 + 32, k_col,
                                              mt * 128:(mt + 1) * 128],
                                        q_ap,
                                        start=True, stop=True,
                                        tile_position=(kp0, 0),
                                    )
                            p_t = pp.tile([128, 1536], f16)
                            p2_t = p2p.tile([128, 1536], f16)
                            nc.scalar.activation(p_t[:], s_t[:], AF.Exp)
                            nc.vector.tensor_mul(
                                p2_t[:],
                                p_t[:],
                                emr_t[:, g * 3:g * 3 + 3],
                            )
                            # PV: O[n, 33] accumulated over mt
                            for hl in range(3):
                                h = g * 3 + hl
                                for nt in range(2):
                                    for mt in range(2):
                                        nc.tensor.matmul(
                                            o_t[:, nt * 256 + h * 33:
                                                nt * 256 + h * 33 + 33],
                                            p2_t[:, hl * 512 + mt * 256 +
                                                 nt * 128:
                                                 hl * 512 + mt * 256 +
                                                 nt * 128 + 128],
                                            v_t[:, mt, h * 33:h * 33 + 33],
                                            start=(mt == 0), stop=(mt == 1),
                                        )
                        # normalize: out[:, h*32+d] = O[:, h*33+d] / O[:, h*33+32]
                        rec_t = recp.tile([128, 2, 6], f32)
                        denom_ap = bass.AP(
                            tensor=o_t.tensor, offset=o_t.offset + 32,
                            ap=[o_t.ap[0], [256, 2], [33, 6]])
                        nc.vector.reciprocal(rec_t[:], denom_ap)
                        out_t = outp.tile([128, 2, 192], f32)
                        for nt in range(2):
                            num_ap = bass.AP(
                                tensor=o_t.tensor,
                                offset=o_t.offset + nt * 256,
                                ap=[o_t.ap[0], [33, 6], [1, 32]])
                            recb_ap = bass.AP(
                                tensor=rec_t.tensor,
                                offset=rec_t.offset + nt * 6,
                                ap=[rec_t.ap[0], [1, 6], [0, 32]])
                            nc.vector.tensor_mul(out_t[:, nt], num_ap, recb_ap)
                        nc.default_dma_engine.dma_start(out=out_d[b, w],
                                                        in_=out_t[:])
    nc.compile()
    return nc


def _get_nc():
    if "nc" not in _NC_CACHE:
        _NC_CACHE["nc"] = _build_nc()
    return _NC_CACHE["nc"]


def _prep_core_inputs(core, qkv, em, ert_np):
    """Build the per-core input dict (everything except replicated ert)."""
    lo = core * W_PER_CORE * N  # token offset (16 H-rows * 256 cols)
    qkv_c = qkv[:, :, lo:lo + W_PER_CORE * N, :]
    # [3, b, hi2, r, wi, cc, h, d]
    x = qkv_c.reshape(3, B, 2, 8, 8, 32, HEADS, HD)
    # -> [w(hi2,wi), b, h, d, n(r,cc)]
    xt = np.ascontiguousarray(x.transpose(0, 2, 4, 1, 6, 7, 3, 5)).reshape(
        3, W_PER_CORE, B, HEADS, HD, 256)
    q = xt[0] * SCALE
    k = xt[1]

    qk_main = np.empty((W_PER_CORE, B, 128, 2, 256), np.float16)
    qk_main[:, :, :, 0, :] = q[:, :, :4].reshape(W_PER_CORE, B, 128, 256)
    qk_main[:, :, :, 1, :] = k[:, :, :4].reshape(W_PER_CORE, B, 128, 256)

    # tail: heads 4,5; partition = bl*64 + (h-4)*32 + d, per (w, bpair)
    qk_tail = np.empty((W_PER_CORE, 2, 128, 2, 256), np.float16)
    qt = q[:, :, 4:].reshape(W_PER_CORE, 2, 2, 64, 256)  # [w, bp, bl, (h2 d), n]
    kt = k[:, :, 4:].reshape(W_PER_CORE, 2, 2, 64, 256)
    qk_tail[:, :, :, 0, :] = qt.reshape(W_PER_CORE, 2, 128, 256)
    qk_tail[:, :, :, 1, :] = kt.reshape(W_PER_CORE, 2, 128, 256)

    # v_aug: [w, b, p(m within tile), mt, h*33+j]; col 32 = 1.0
    v = np.ascontiguousarray(x[2].transpose(1, 3, 0, 2, 4, 5, 6)).reshape(
        W_PER_CORE, B, 256, HEADS, HD)  # [w, b, m(r,cc), h, d]
    vaug = np.empty((W_PER_CORE, B, 2, 128, HEADS, 33), np.float16)
    vaug[..., :32] = v.reshape(W_PER_CORE, B, 2, 128, HEADS, HD)
    vaug[..., 32] = 1.0
    vaug = vaug.reshape(W_PER_CORE, B, 2, 128, 198).transpose(0, 1, 3, 2, 4)
    vaug = np.ascontiguousarray(vaug)

    # emt: per-core mask slice -> [w, p(m), mt, n]
    em_c = em[core * W_PER_CORE:(core + 1) * W_PER_CORE]  # [w, n, m] fp32
    emt = em_c.transpose(0, 2, 1).reshape(W_PER_CORE, 2, 128, 256)
    emt = np.ascontiguousarray(emt.transpose(0, 2, 1, 3)).astype(np.float16)

    return {
        "qk_main": qk_main,
        "qk_tail": qk_tail,
        "v_aug": vaug.astype(np.float16),
        "emt": emt,
        "ert": ert_np,
    }


def kernel(qkv, mask, rpe_biases, pw0, pb0, g1, be1, w1, b1, g2, be2, w2, b2,
           g3, be3, w3, b3, rpi, H, W, **_unused):
    qkv = np.asarray(qkv, np.float32)
    mask = np.asarray(mask, np.float32)
    rpi = np.asarray(rpi).astype(np.int64)

    pos = _pos_mlp_host(
        np.asarray(rpe_biases, np.float32), np.asarray(pw0, np.float32),
        np.asarray(pb0, np.float32), np.asarray(g1, np.float32),
        np.asarray(be1, np.float32), np.asarray(w1, np.float32),
        np.asarray(b1, np.float32), np.asarray(g2, np.float32),
        np.asarray(be2, np.float32), np.asarray(w2, np.float32),
        np.asarray(b2, np.float32), np.asarray(g3, np.float32),
        np.asarray(be3, np.float32), np.asarray(w3, np.float32),
        np.asarray(b3, np.float32))
    rpb = pos[rpi.reshape(-1)].reshape(N, N, HEADS)  # [n, m, h]

    # ert[p, mt, h, n] = exp(rpb[n, mt*128+p, h])
    er = np.exp(rpb).transpose(1, 2, 0)  # [m, h, n]
    ert_np = np.ascontiguousarray(
        er.reshape(2, 128, HEADS, 256).transpose(1, 2, 0, 3)).astype(np.float16)

    em = np.exp(mask) * PRESCALE  # [w, n, m] fp32

    in_maps = [_prep_core_inputs(c, qkv, em, ert_np) for c in range(N_CORES)]

    from concourse.bass_utils import run_bass_kernel_spmd
    nc = _get_nc()
    res = run_bass_kernel_spmd(nc, in_maps, core_ids=list(range(N_CORES)))
    _NC_CACHE["last_results"] = res

    # gather: out_dev (B, 16, 128, 2, 192) per core -> (B, H, W, C)
    out = np.empty((B, H_FULL, W_FULL, C), np.float32)
    for c in range(N_CORES):
        o = res.results[c]["out"]  # [b, w, p, nt, ch]
        o = o.transpose(0, 1, 3, 2, 4).reshape(B, 2, 8, 8, 32, C)
        # [b, hi2, wi, r, cc, ch] -> rows
        o = o.transpose(0, 1, 3, 2, 4, 5).reshape(B, 16, 256, C)
        out[:, c * 16:(c + 1) * 16] = o
    return out
